# revision 36
# baseline (speedup 1.0000x reference)
"""DeepSpeed-style MLP block (residual-add + LayerNorm + GEMM + GeLU + GEMM +
residual) on 8 Trainium2 NeuronCores.

Sharding: data-parallel over tokens (B*S = 8192 -> 1024 tokens/core).  Each
core holds the full weights and computes its token slice end-to-end; no
collectives.

All matmuls run in bf16 on the PE (1 cycle/row vs fp32's 4) with fp32 PSUM
accumulation.  Per core the tokens are processed as two 512-token groups:
GEMM1 (64 rank-128 i-chunks) produces h^T tiles [128, 512] that stay resident
in SBUF, then GEMM2 accumulates over all 64 i-chunks into PSUM for 4 output
column chunks of 512.  Weights stream from DRAM twice (once per group), which
the DMA engines hide entirely under the PE's compute.

LayerNorm statistics use bn_stats/bn_aggr on the vector engine.  gamma/beta
are folded into W1/b1 host-side (ln@W1+b1 == z@(gamma*W1) + (beta@W1+b1)),
so the [tok,H] -> [H,tok] PE transposes drain PSUM->SBUF as plain copies,
four 128-column chunks per activation instruction.  GEMM1 starts on a
256-token sub-pass as soon as the first two token tiles' LayerNorm is done,
and GEMM2's PSUM accumulators rotate through six tag slots so column passes
never wait on drains.
"""

import sys

sys.path.insert(0, "/opt/trn_rl_repo")

import numpy as np

try:
    import jax

    jax.config.update("jax_compilation_cache_dir", "/tmp/jax_neff_cache")
    jax.config.update("jax_persistent_cache_min_compile_time_secs", 1.0)
    jax.config.update("jax_persistent_cache_min_entry_size_bytes", 0)
except Exception:
    pass

import ml_dtypes

import concourse.bass as bass  # noqa: F401
import concourse.mybir as mybir
from concourse import bacc
from concourse.masks import make_identity
from concourse.tile import TileContext

F32 = mybir.dt.float32
BF16 = mybir.dt.bfloat16
AF = mybir.ActivationFunctionType
ALU = mybir.AluOpType
NP_BF16 = ml_dtypes.bfloat16

N_CORES = 8
B, S, H, I = 4, 2048, 2048, 8192
LN_EPS = 1e-6
NTOK = B * S                 # 8192 tokens total
TLOC = NTOK // N_CORES       # 1024 tokens per core
TT = TLOC // 128             # 8 token tiles per core
HC = H // 128                # 16 hidden chunks (contraction for GEMM1)
IC = I // 128                # 64 intermediate chunks
GROUPS = 2                   # token groups per core
GT = TT // GROUPS            # 4 token tiles per group
GTOK = TLOC // GROUPS        # 512 tokens per group
OC = H // 512                # 4 output column chunks of 512

_CACHE = {}


def _build_program():
    nc = bacc.Bacc("TRN2", target_bir_lowering=False, debug=False,
                   num_devices=N_CORES)

    xin = nc.declare_dram_parameter("xin", [TLOC, H], BF16, isOutput=False)
    xres = nc.declare_dram_parameter("xres", [TLOC, H], BF16, isOutput=False)
    # w1p[i, p, c*128 + f] = (gamma[:, None] * inter_w)[c*128 + p, i*128 + f]
    # (LayerNorm's gamma/beta are folded into W1/b1 host-side)
    w1p = nc.declare_dram_parameter("w1p", [IC, 128, H], BF16, isOutput=False)
    w2p = nc.declare_dram_parameter("w2p", [I, H], BF16, isOutput=False)
    bbt = nc.declare_dram_parameter("bbt", [128, H], BF16, isOutput=False)
    obt = nc.declare_dram_parameter("obt", [128, H], BF16, isOutput=False)
    # b1t[p, i] = (beta @ inter_w + inter_b)[i*128 + p]
    b1t = nc.declare_dram_parameter("b1t", [128, IC], F32, isOutput=False)
    out = nc.declare_dram_parameter("out", [TLOC, H], F32, isOutput=True)

    with TileContext(nc) as tc:
        with (
            tc.tile_pool(name="perm", bufs=1) as perm,
            tc.tile_pool(name="p1", bufs=2) as p1,
            tc.tile_pool(name="w1pool", bufs=4) as w1pool,
            tc.tile_pool(name="w2pool", bufs=6) as w2pool,
            tc.tile_pool(name="htpool", bufs=IC) as htpool,
            tc.tile_pool(name="osbp", bufs=4) as osbp,
            tc.tile_pool(name="ps", bufs=1, space="PSUM") as ps,
        ):
            ident = perm.tile([128, 128], BF16)
            eps = perm.tile([128, 1], F32)
            b1 = perm.tile([128, IC], F32)
            bb = perm.tile([128, H], BF16)
            ob = perm.tile([128, H], BF16)

            # ln^T, chunk-major: lnt[:, c, tok] = ln[tok, c*128 + p]
            lnta = perm.tile([128, HC, TLOC], BF16, name="lnta")
            lnt = lnta[:]
            rao = [perm.tile([128, H], BF16, name=f"rao{t}")
                   for t in range(TT)]

            zs = {}

            def p1_load(t, split=False):
                tin = p1.tile([128, H], BF16, tag="tin")
                tre = p1.tile([128, H], BF16, tag="tre")
                # issue the first tile's two loads on different queues so the
                # transfers overlap (the LayerNorm chain start gates the PE)
                eng = nc.scalar if split else nc.sync
                nc.sync.dma_start(out=tin[:], in_=xin[t * 128:(t + 1) * 128, :])
                eng.dma_start(out=tre[:], in_=xres[t * 128:(t + 1) * 128, :])
                return tin, tre

            def p1_compute(t, loaded=None):
                """residual add + LN stats + normalize for token tile t."""
                tin, tre = loaded if loaded is not None else p1_load(t)
                ra = p1.tile([128, H], BF16, tag="ra")
                nc.vector.tensor_add(ra[:], tin[:], tre[:])
                nc.vector.tensor_add(ra[:], ra[:], bb[:])
                # final-residual term (ra + output_b), off the critical path
                nc.gpsimd.tensor_add(rao[t][:], ra[:], ob[:])
                # mean/var via bn_stats over 4 chunks of 512
                stats = p1.tile([128, 4, 6], F32, tag="stats")
                rav = ra[:].rearrange("p (n f) -> p n f", f=512)
                for sub in range(4):
                    nc.vector.bn_stats(stats[:, sub, :], rav[:, sub, :])
                mv = p1.tile([128, 2], F32, tag="mv")
                nc.vector.bn_aggr(mv[:], stats[:])
                std = p1.tile([128, 1], F32, tag="std")
                nc.scalar.activation(std[:], mv[:, 1:2], AF.Sqrt, bias=eps[:])
                rstd = p1.tile([128, 1], F32, tag="rstd")
                nc.vector.reciprocal(rstd[:], std[:])
                z = p1.tile([128, H], BF16, tag="z", bufs=4)
                nc.vector.tensor_scalar(
                    z[:], ra[:], mv[:, 0:1], rstd[:],
                    op0=ALU.subtract, op1=ALU.mult)
                zs[t] = z

            def p1_transpose(t):
                """z[tok, H] -> lnt[:, c, tok]; 4 chunks per PSUM drain."""
                z = zs[t]
                for cq in range(HC // 4):
                    tr = ps.tile([128, 512], BF16, tag="trp", bufs=2)
                    for j in range(4):
                        c = cq * 4 + j
                        nc.tensor.transpose(
                            tr[:, j * 128:(j + 1) * 128],
                            z[:, c * 128:(c + 1) * 128], ident[:])
                    trv = tr[:].rearrange("p (n f) -> p n f", f=128)
                    nc.scalar.activation(
                        lnt[:, cq * 4:(cq + 1) * 4, t * 128:(t + 1) * 128],
                        trv, AF.Copy)

            hts = [[None] * IC for _ in range(GROUPS)]

            def w1_load(i):
                w1t = w1pool.tile([128, H], BF16, tag="w1t")
                nc.sync.dma_start(out=w1t[:], in_=w1p[i])
                return w1t

            def g1_chunk(g, i, w1t=None, sub=None):
                """h^T[i-block] = gelu(W1^T @ ln^T + b1) for group g.

                sub=None computes all GTOK tokens; sub=0/1 computes the
                first/second 256-token half (used to start the PE before
                the later token tiles' LayerNorm has finished).
                """
                if w1t is None:
                    w1t = w1_load(i)
                if sub is None:
                    lo, n = 0, GTOK
                elif isinstance(sub, tuple):
                    lo, n = sub
                else:
                    lo, n = sub * (GTOK // 2), GTOK // 2
                psh = ps.tile([128, GTOK], F32, tag="psh", bufs=2)
                for c in range(HC):
                    nc.tensor.matmul(
                        psh[:, :n],
                        w1t[:, c * 128:(c + 1) * 128],
                        lnt[:, c, g * GTOK + lo:g * GTOK + lo + n],
                        start=(c == 0), stop=(c == HC - 1))
                if lo == 0:
                    ht = htpool.tile([128, GTOK], BF16, tag="ht")
                    hts[g][i] = ht
                nc.scalar.activation(hts[g][i][:, lo:lo + n], psh[:, :n],
                                     AF.Gelu, bias=b1[:, i:i + 1])

            def w2_load(oc, i):
                w2c = w2pool.tile([128, 512], BF16, tag="w2c")
                nc.scalar.dma_start(
                    out=w2c[:],
                    in_=w2p[i * 128:(i + 1) * 128, oc * 512:(oc + 1) * 512])
                return w2c

            # GEMM2 PSUM accumulators rotate through 6 tag slots (4 dedicated
            # + the GEMM1/transpose banks, idle during a GEMM2 pass) so a new
            # column pass never waits on the previous pass's drains.
            pso_slots = [("pso0", 1), ("pso1", 1), ("pso2", 1), ("pso3", 1),
                         ("psh", 2), ("trp", 2)]
            pso_cnt = [0]

            def g2_group(g, preloaded=()):
                """out[group tokens] = h @ W2 + (ra + output_b)."""
                for oc in range(OC):
                    psos = []
                    for t in range(GT):
                        tag, nb = pso_slots[(pso_cnt[0] + t) % len(pso_slots)]
                        psos.append(ps.tile([128, 512], F32,
                                            name=f"pso_{g}_{oc}_{t}",
                                            tag=tag, bufs=nb))
                    pso_cnt[0] += GT
                    last = (g == GROUPS - 1) and (oc == OC - 1)
                    o_lo = oc * 512

                    def drain(t, eng):
                        tt = g * GT + t
                        osb = osbp.tile([128, 512], F32, tag="osb",
                                        name="osb")
                        nc.vector.tensor_add(
                            osb[:], psos[t][:], rao[tt][:, o_lo:o_lo + 512])
                        eng.dma_start(
                            out=out[tt * 128:(tt + 1) * 128,
                                    o_lo:o_lo + 512],
                            in_=osb[:])

                    for i in range(IC):
                        if oc == 0 and i < len(preloaded):
                            w2c = preloaded[i]
                        else:
                            w2c = w2_load(oc, i)
                        ht = hts[g][i]
                        for t in range(GT):
                            nc.tensor.matmul(
                                psos[t][:],
                                ht[:, t * 128:(t + 1) * 128],
                                w2c[:],
                                start=(i == 0), stop=(i == IC - 1))
                            if last and i == IC - 1:
                                # flush each tile as soon as it stops so the
                                # final drain tail is short
                                drain(t, nc.sync if t % 2 == 0 else nc.scalar)
                    if not last:
                        for t in range(GT):
                            drain(t, nc.sync)

            # ---- emission order: pipeline phase 1 under GEMM1 of group 0 ----
            # DMA order puts tile 0/1 activations first so the LayerNorm
            # chain (the critical path to the first matmul) starts ASAP.
            NSUB = 16   # leading GEMM1 i-chunks run as two 256-token passes
            l0 = p1_load(0, split=True)
            nc.sync.dma_start(out=bb[:], in_=bbt[:])
            l1 = p1_load(1, split=True)
            nc.sync.dma_start(out=ob[:], in_=obt[:])
            nc.sync.dma_start(out=b1[:], in_=b1t[:])
            make_identity(nc, ident[:])
            nc.vector.memset(eps[:], LN_EPS)
            with tc.high_priority():
                p1_compute(0, l0)
                p1_compute(1, l1)
                p1_transpose(0)
                p1_transpose(1)
            p1_compute(2)
            p1_compute(3)

            # first NSUB chunks: tokens 0-255 only (needs just tiles 0-1), so
            # the PE starts as soon as the first two LayerNorm tiles are done
            w1_first = [w1_load(i) for i in range(min(3, NSUB))]
            for i in range(0, NSUB):
                g1_chunk(0, i, w1t=w1_first[i] if i < len(w1_first) else None,
                         sub=0)
            # prefetch the re-loads for the second 256-token pass
            w1_sub1 = [w1_load(i) for i in range(2)]
            p1_transpose(2)
            p1_transpose(3)
            p1_compute(4)
            # second half of the leading chunks (tokens 256-511)
            for i in range(0, NSUB):
                g1_chunk(0, i, w1t=w1_sub1[i] if i < len(w1_sub1) else None,
                         sub=1)
            p1_compute(5)
            for i in range(NSUB, 16):
                g1_chunk(0, i)
            p1_transpose(4)
            p1_compute(6)
            for i in range(16, 24):
                g1_chunk(0, i)
            p1_transpose(5)
            p1_compute(7)
            for i in range(24, 32):
                g1_chunk(0, i)
            p1_transpose(6)
            for i in range(32, 40):
                g1_chunk(0, i)
            p1_transpose(7)
            for i in range(40, IC - 8):
                g1_chunk(0, i)
            # prefetch the first W2 column chunks (ACT queue) so GEMM2 starts
            # seamlessly after GEMM1's last chunk
            w2_first = [w2_load(0, i) for i in range(4)]
            for i in range(IC - 8, IC):
                g1_chunk(0, i)

            g2_group(0, preloaded=w2_first)
            for i in range(IC - 8):
                g1_chunk(1, i)
            w2_g1 = [w2_load(0, i) for i in range(4)]
            for i in range(IC - 8, IC):
                g1_chunk(1, i)
            g2_group(1, preloaded=w2_g1)

    nc.compile()
    return nc


def _get_program():
    if "nc" not in _CACHE:
        _CACHE["nc"] = _build_program()
    return _CACHE["nc"]


def kernel(input, residual, residual_norm, bias, gamma, beta,
           inter_w, inter_b, output_w, output_b):
    nc = _get_program()

    input = np.asarray(input, dtype=np.float32)
    residual = np.asarray(residual, dtype=np.float32)
    bias = np.asarray(bias, dtype=np.float32)
    gamma = np.asarray(gamma, dtype=np.float32)
    beta = np.asarray(beta, dtype=np.float32)
    inter_w = np.asarray(inter_w, dtype=np.float32)
    inter_b = np.asarray(inter_b, dtype=np.float32)
    output_w = np.asarray(output_w, dtype=np.float32)
    output_b = np.asarray(output_b, dtype=np.float32)

    xin = np.ascontiguousarray(input.reshape(NTOK, H).astype(NP_BF16))
    xres = np.ascontiguousarray(residual.reshape(NTOK, H).astype(NP_BF16))
    # fold LayerNorm's gamma/beta into W1/b1:
    #   ln @ W1 + b1 == z @ (gamma[:,None]*W1) + (beta @ W1 + b1)
    w1f = gamma[:, None].astype(np.float32) * inter_w
    b1f = beta.astype(np.float32) @ inter_w + inter_b
    # w1p[i, p, c*128+f] = w1f[c*128+p, i*128+f]
    w1p = np.ascontiguousarray(
        w1f.reshape(HC, 128, IC, 128).transpose(2, 1, 0, 3)
        .reshape(IC, 128, H).astype(NP_BF16))
    w2p = np.ascontiguousarray(output_w.astype(NP_BF16))
    bbt = np.ascontiguousarray(
        np.broadcast_to(bias.astype(NP_BF16), (128, H)))
    obt = np.ascontiguousarray(
        np.broadcast_to(output_b.astype(NP_BF16), (128, H)))
    b1t = np.ascontiguousarray(b1f.reshape(IC, 128).T)

    in_maps = []
    for c in range(N_CORES):
        in_maps.append({
            "xin": np.ascontiguousarray(xin[c * TLOC:(c + 1) * TLOC]),
            "xres": np.ascontiguousarray(xres[c * TLOC:(c + 1) * TLOC]),
            "w1p": w1p,
            "w2p": w2p,
            "bbt": bbt,
            "obt": obt,
            "b1t": b1t,
        })

    from concourse.bass_utils import run_bass_kernel_spmd
    res = run_bass_kernel_spmd(nc, in_maps, list(range(N_CORES)))
    out = np.concatenate([res.results[c]["out"] for c in range(N_CORES)],
                         axis=0)
    return out.reshape(B, S, H)


if __name__ == "__main__":
    nc = _get_program()
    from concourse.timeline_sim import TimelineSim
    ts = TimelineSim(nc)
    total = ts.simulate()
    print(f"TimelineSim: {total:.0f} ns")


# revision 48
# speedup vs baseline: 1.0003x; 1.0003x over previous
"""DeepSpeed-style MLP block (residual-add + LayerNorm + GEMM + GeLU + GEMM +
residual) on 8 Trainium2 NeuronCores.

Sharding: data-parallel over tokens (B*S = 8192 -> 1024 tokens/core).  Each
core holds the full weights and computes its token slice end-to-end; no
collectives.

All matmuls run in bf16 on the PE (1 cycle/row vs fp32's 4) with fp32 PSUM
accumulation.  Per core the tokens are processed as two 512-token groups:
GEMM1 (64 rank-128 i-chunks) produces h^T tiles [128, 512] that stay resident
in SBUF, then GEMM2 accumulates over all 64 i-chunks into PSUM for 4 output
column chunks of 512.  Weights stream from DRAM twice (once per group), which
the DMA engines hide entirely under the PE's compute.

LayerNorm statistics use bn_stats/bn_aggr on the vector engine.  gamma/beta
are folded into W1/b1 host-side (ln@W1+b1 == z@(gamma*W1) + (beta@W1+b1)),
so the [tok,H] -> [H,tok] PE transposes drain PSUM->SBUF as plain copies,
four 128-column chunks per activation instruction.  GEMM1 starts on a
256-token sub-pass as soon as the first two token tiles' LayerNorm is done,
and GEMM2's PSUM accumulators rotate through six tag slots so column passes
never wait on drains.
"""

import sys

sys.path.insert(0, "/opt/trn_rl_repo")

import numpy as np

try:
    import jax

    jax.config.update("jax_compilation_cache_dir", "/tmp/jax_neff_cache")
    jax.config.update("jax_persistent_cache_min_compile_time_secs", 1.0)
    jax.config.update("jax_persistent_cache_min_entry_size_bytes", 0)
except Exception:
    pass

import ml_dtypes

import concourse.bass as bass  # noqa: F401
import concourse.mybir as mybir
from concourse import bacc
from concourse.masks import make_identity
from concourse.tile import TileContext

F32 = mybir.dt.float32
BF16 = mybir.dt.bfloat16
AF = mybir.ActivationFunctionType
ALU = mybir.AluOpType
NP_BF16 = ml_dtypes.bfloat16
U32 = mybir.dt.uint32

N_CORES = 8
B, S, H, I = 4, 2048, 2048, 8192
LN_EPS = 1e-6
NTOK = B * S                 # 8192 tokens total
TLOC = NTOK // N_CORES       # 1024 tokens per core
TT = TLOC // 128             # 8 token tiles per core
HC = H // 128                # 16 hidden chunks (contraction for GEMM1)
IC = I // 128                # 64 intermediate chunks
GROUPS = 2                   # token groups per core
GT = TT // GROUPS            # 4 token tiles per group
GTOK = TLOC // GROUPS        # 512 tokens per group
OC = H // 512                # 4 output column chunks of 512

_CACHE = {}


def _build_program():
    nc = bacc.Bacc("TRN2", target_bir_lowering=False, debug=False,
                   num_devices=N_CORES)

    xin = nc.declare_dram_parameter("xin", [TLOC, H], BF16, isOutput=False)
    xres = nc.declare_dram_parameter("xres", [TLOC, H], BF16, isOutput=False)
    # w1p[i, p, c*128 + f] = (gamma[:, None] * inter_w)[c*128 + p, i*128 + f]
    # (LayerNorm's gamma/beta are folded into W1/b1 host-side)
    w1p = nc.declare_dram_parameter("w1p", [IC, 128, H], BF16, isOutput=False)
    w2p = nc.declare_dram_parameter("w2p", [I, H], BF16, isOutput=False)
    bbt = nc.declare_dram_parameter("bbt", [128, H], BF16, isOutput=False)
    obt = nc.declare_dram_parameter("obt", [128, H], BF16, isOutput=False)
    # b1t[p, i] = (beta @ inter_w + inter_b)[i*128 + p]
    b1t = nc.declare_dram_parameter("b1t", [128, IC], F32, isOutput=False)
    out = nc.declare_dram_parameter("out", [TLOC, H], F32, isOutput=True)

    with TileContext(nc) as tc:
        with (
            tc.tile_pool(name="perm", bufs=1) as perm,
            tc.tile_pool(name="p1", bufs=2) as p1,
            tc.tile_pool(name="w1pool", bufs=4) as w1pool,
            tc.tile_pool(name="w2pool", bufs=6) as w2pool,
            tc.tile_pool(name="htpool", bufs=IC) as htpool,
            tc.tile_pool(name="osbp", bufs=4) as osbp,
            tc.tile_pool(name="ps", bufs=1, space="PSUM") as ps,
        ):
            ident = perm.tile([128, 128], BF16)
            eps = perm.tile([128, 1], F32)
            b1 = perm.tile([128, IC], F32)
            bb = perm.tile([128, H], BF16)
            ob = perm.tile([128, H], BF16)

            # ln^T, chunk-major: lnt[:, c, tok] = ln[tok, c*128 + p]
            lnta = perm.tile([128, HC, TLOC], BF16, name="lnta")
            lnt = lnta[:]
            rao = [perm.tile([128, H], BF16, name=f"rao{t}")
                   for t in range(TT)]

            zs = {}

            HH = H // 2

            def p1_load(t, split=False):
                tin = p1.tile([128, H], BF16, tag="tin")
                tre = p1.tile([128, H], BF16, tag="tre")
                rows = slice(t * 128, (t + 1) * 128)
                if split:
                    # half-tile loads alternating SP/ACT queues: compute can
                    # start after half the bytes have landed (the LayerNorm
                    # chain start gates the PE)
                    nc.sync.dma_start(out=tin[:, :HH], in_=xin[rows, :HH])
                    nc.scalar.dma_start(out=tre[:, :HH], in_=xres[rows, :HH])
                    nc.scalar.dma_start(out=tin[:, HH:], in_=xin[rows, HH:])
                    nc.sync.dma_start(out=tre[:, HH:], in_=xres[rows, HH:])
                else:
                    nc.sync.dma_start(out=tin[:], in_=xin[rows, :])
                    nc.sync.dma_start(out=tre[:], in_=xres[rows, :])
                return tin, tre

            def p1_compute(t, loaded=None, split=False):
                """residual add + LN stats + normalize for token tile t."""
                tin, tre = loaded if loaded is not None else p1_load(t)
                ra = p1.tile([128, H], BF16, tag="ra")
                if split:
                    for h in (slice(0, HH), slice(HH, H)):
                        nc.vector.tensor_add(ra[:, h], tin[:, h], tre[:, h])
                        nc.vector.tensor_add(ra[:, h], ra[:, h], bb[:, h])
                else:
                    nc.vector.tensor_add(ra[:], tin[:], tre[:])
                    nc.vector.tensor_add(ra[:], ra[:], bb[:])
                # final-residual term (ra + output_b), off the critical path
                nc.gpsimd.tensor_add(rao[t][:], ra[:], ob[:])
                # mean/var via bn_stats over 4 chunks of 512
                stats = p1.tile([128, 4, 6], F32, tag="stats")
                rav = ra[:].rearrange("p (n f) -> p n f", f=512)
                for sub in range(4):
                    nc.vector.bn_stats(stats[:, sub, :], rav[:, sub, :])
                mv = p1.tile([128, 2], F32, tag="mv")
                nc.vector.bn_aggr(mv[:], stats[:])
                std = p1.tile([128, 1], F32, tag="std")
                nc.scalar.activation(std[:], mv[:, 1:2], AF.Sqrt,
                                     bias=eps[:])
                rstd = p1.tile([128, 1], F32, tag="rstd")
                nc.vector.reciprocal(rstd[:], std[:])
                z = p1.tile([128, H], BF16, tag="z", bufs=4)
                nc.vector.tensor_scalar(
                    z[:], ra[:], mv[:, 0:1], rstd[:],
                    op0=ALU.subtract, op1=ALU.mult)
                zs[t] = z

            def p1_transpose(t):
                """z[tok, H] -> lnt[:, c, tok]; 4 chunks per PSUM drain."""
                z = zs[t]
                for cq in range(HC // 4):
                    tr = ps.tile([128, 512], BF16, tag="trp", bufs=2)
                    for j in range(4):
                        c = cq * 4 + j
                        nc.tensor.transpose(
                            tr[:, j * 128:(j + 1) * 128],
                            z[:, c * 128:(c + 1) * 128], ident[:])
                    trv = tr[:].rearrange("p (n f) -> p n f", f=128)
                    nc.scalar.activation(
                        lnt[:, cq * 4:(cq + 1) * 4, t * 128:(t + 1) * 128],
                        trv, AF.Copy)

            hts = [[None] * IC for _ in range(GROUPS)]

            def w1_load(i):
                w1t = w1pool.tile([128, H], BF16, tag="w1t")
                nc.sync.dma_start(out=w1t[:], in_=w1p[i])
                return w1t

            def g1_chunk(g, i, w1t=None, sub=None):
                """h^T[i-block] = gelu(W1^T @ ln^T + b1) for group g.

                sub=None computes all GTOK tokens; sub=0/1 computes the
                first/second 256-token half (used to start the PE before
                the later token tiles' LayerNorm has finished).
                """
                if w1t is None:
                    w1t = w1_load(i)
                if sub is None:
                    lo, n = 0, GTOK
                elif isinstance(sub, tuple):
                    lo, n = sub
                else:
                    lo, n = sub * (GTOK // 2), GTOK // 2
                psh = ps.tile([128, GTOK], F32, tag="psh", bufs=2)
                for c in range(HC):
                    nc.tensor.matmul(
                        psh[:, :n],
                        w1t[:, c * 128:(c + 1) * 128],
                        lnt[:, c, g * GTOK + lo:g * GTOK + lo + n],
                        start=(c == 0), stop=(c == HC - 1))
                if lo == 0:
                    ht = htpool.tile([128, GTOK], BF16, tag="ht")
                    hts[g][i] = ht
                nc.scalar.activation(hts[g][i][:, lo:lo + n], psh[:, :n],
                                     AF.Gelu, bias=b1[:, i:i + 1])

            def w2_load(oc, i):
                w2c = w2pool.tile([128, 512], BF16, tag="w2c")
                nc.scalar.dma_start(
                    out=w2c[:],
                    in_=w2p[i * 128:(i + 1) * 128, oc * 512:(oc + 1) * 512])
                return w2c

            # GEMM2 PSUM accumulators rotate through 6 tag slots (4 dedicated
            # + the GEMM1/transpose banks, idle during a GEMM2 pass) so a new
            # column pass never waits on the previous pass's drains.
            pso_slots = [("pso0", 1), ("pso1", 1), ("pso2", 1), ("pso3", 1),
                         ("psh", 2), ("trp", 2)]
            pso_cnt = [0]

            def g2_group(g, preloaded=()):
                """out[group tokens] = h @ W2 + (ra + output_b)."""
                for oc in range(OC):
                    psos = []
                    for t in range(GT):
                        tag, nb = pso_slots[(pso_cnt[0] + t) % len(pso_slots)]
                        psos.append(ps.tile([128, 512], F32,
                                            name=f"pso_{g}_{oc}_{t}",
                                            tag=tag, bufs=nb))
                    pso_cnt[0] += GT
                    last = (g == GROUPS - 1) and (oc == OC - 1)
                    o_lo = oc * 512

                    def drain(t, eng):
                        tt = g * GT + t
                        osb = osbp.tile([128, 512], F32, tag="osb",
                                        name="osb")
                        nc.vector.tensor_add(
                            osb[:], psos[t][:], rao[tt][:, o_lo:o_lo + 512])
                        eng.dma_start(
                            out=out[tt * 128:(tt + 1) * 128,
                                    o_lo:o_lo + 512],
                            in_=osb[:])

                    for i in range(IC):
                        if oc == 0 and i < len(preloaded):
                            w2c = preloaded[i]
                        else:
                            w2c = w2_load(oc, i)
                        ht = hts[g][i]
                        for t in range(GT):
                            nc.tensor.matmul(
                                psos[t][:],
                                ht[:, t * 128:(t + 1) * 128],
                                w2c[:],
                                start=(i == 0), stop=(i == IC - 1))
                            if last and i == IC - 1:
                                # flush each tile as soon as it stops so the
                                # final drain tail is short
                                drain(t, nc.sync if t % 2 == 0 else nc.scalar)
                    if not last:
                        for t in range(GT):
                            drain(t, nc.sync)

            # ---- emission order: pipeline phase 1 under GEMM1 of group 0 ----
            # DMA order puts tile 0/1 activations first so the LayerNorm
            # chain (the critical path to the first matmul) starts ASAP.
            NSUB = 16   # leading GEMM1 i-chunks run as two 256-token passes
            l0 = p1_load(0, split=True)
            nc.sync.dma_start(out=bb[:, :HH], in_=bbt[:, :HH])
            nc.scalar.dma_start(out=bb[:, HH:], in_=bbt[:, HH:])
            l1 = p1_load(1, split=True)
            nc.sync.dma_start(out=ob[:], in_=obt[:])
            nc.sync.dma_start(out=b1[:], in_=b1t[:])
            make_identity(nc, ident[:])
            nc.vector.memset(eps[:], LN_EPS)
            with tc.high_priority():
                p1_compute(0, l0, split=True)
                p1_compute(1, l1, split=True)
                p1_transpose(0)
                p1_transpose(1)
            p1_compute(2)
            p1_compute(3)

            # first NSUB chunks: tokens 0-255 only (needs just tiles 0-1), so
            # the PE starts as soon as the first two LayerNorm tiles are done
            w1_first = [w1_load(i) for i in range(min(3, NSUB))]
            for i in range(0, NSUB):
                g1_chunk(0, i, w1t=w1_first[i] if i < len(w1_first) else None,
                         sub=0)
            # prefetch the re-loads for the second 256-token pass
            w1_sub1 = [w1_load(i) for i in range(2)]
            p1_transpose(2)
            p1_transpose(3)
            p1_compute(4)
            # second half of the leading chunks (tokens 256-511)
            for i in range(0, NSUB):
                g1_chunk(0, i, w1t=w1_sub1[i] if i < len(w1_sub1) else None,
                         sub=1)
            p1_compute(5)
            for i in range(NSUB, 16):
                g1_chunk(0, i)
            p1_transpose(4)
            p1_compute(6)
            for i in range(16, 24):
                g1_chunk(0, i)
            p1_transpose(5)
            p1_compute(7)
            for i in range(24, 32):
                g1_chunk(0, i)
            p1_transpose(6)
            for i in range(32, 40):
                g1_chunk(0, i)
            p1_transpose(7)
            for i in range(40, IC - 8):
                g1_chunk(0, i)
            # prefetch the first W2 column chunks (ACT queue) so GEMM2 starts
            # seamlessly after GEMM1's last chunk
            w2_first = [w2_load(0, i) for i in range(4)]
            for i in range(IC - 8, IC):
                g1_chunk(0, i)

            g2_group(0, preloaded=w2_first)
            for i in range(IC - 8):
                g1_chunk(1, i)
            w2_g1 = [w2_load(0, i) for i in range(4)]
            for i in range(IC - 8, IC):
                g1_chunk(1, i)
            g2_group(1, preloaded=w2_g1)

    nc.compile()
    return nc


def _get_program():
    if "nc" not in _CACHE:
        _CACHE["nc"] = _build_program()
    return _CACHE["nc"]


def kernel(input, residual, residual_norm, bias, gamma, beta,
           inter_w, inter_b, output_w, output_b):
    nc = _get_program()

    input = np.asarray(input, dtype=np.float32)
    residual = np.asarray(residual, dtype=np.float32)
    bias = np.asarray(bias, dtype=np.float32)
    gamma = np.asarray(gamma, dtype=np.float32)
    beta = np.asarray(beta, dtype=np.float32)
    inter_w = np.asarray(inter_w, dtype=np.float32)
    inter_b = np.asarray(inter_b, dtype=np.float32)
    output_w = np.asarray(output_w, dtype=np.float32)
    output_b = np.asarray(output_b, dtype=np.float32)

    xin = np.ascontiguousarray(input.reshape(NTOK, H).astype(NP_BF16))
    xres = np.ascontiguousarray(residual.reshape(NTOK, H).astype(NP_BF16))
    # fold LayerNorm's gamma/beta into W1/b1:
    #   ln @ W1 + b1 == z @ (gamma[:,None]*W1) + (beta @ W1 + b1)
    w1f = gamma[:, None].astype(np.float32) * inter_w
    b1f = beta.astype(np.float32) @ inter_w + inter_b
    # w1p[i, p, c*128+f] = w1f[c*128+p, i*128+f]
    w1p = np.ascontiguousarray(
        w1f.reshape(HC, 128, IC, 128).transpose(2, 1, 0, 3)
        .reshape(IC, 128, H).astype(NP_BF16))
    w2p = np.ascontiguousarray(output_w.astype(NP_BF16))
    bbt = np.ascontiguousarray(
        np.broadcast_to(bias.astype(NP_BF16), (128, H)))
    obt = np.ascontiguousarray(
        np.broadcast_to(output_b.astype(NP_BF16), (128, H)))
    b1t = np.ascontiguousarray(b1f.reshape(IC, 128).T)

    in_maps = []
    for c in range(N_CORES):
        in_maps.append({
            "xin": np.ascontiguousarray(xin[c * TLOC:(c + 1) * TLOC]),
            "xres": np.ascontiguousarray(xres[c * TLOC:(c + 1) * TLOC]),
            "w1p": w1p,
            "w2p": w2p,
            "bbt": bbt,
            "obt": obt,
            "b1t": b1t,
        })

    from concourse.bass_utils import run_bass_kernel_spmd
    res = run_bass_kernel_spmd(nc, in_maps, list(range(N_CORES)))
    out = np.concatenate([res.results[c]["out"] for c in range(N_CORES)],
                         axis=0)
    return out.reshape(B, S, H)


if __name__ == "__main__":
    nc = _get_program()
    from concourse.timeline_sim import TimelineSim
    ts = TimelineSim(nc)
    total = ts.simulate()
    print(f"TimelineSim: {total:.0f} ns")


# revision 52
# speedup vs baseline: 1.0015x; 1.0012x over previous
"""DeepSpeed-style MLP block (residual-add + LayerNorm + GEMM + GeLU + GEMM +
residual) on 8 Trainium2 NeuronCores.

Sharding: data-parallel over tokens (B*S = 8192 -> 1024 tokens/core).  Each
core holds the full weights and computes its token slice end-to-end; no
collectives.

All matmuls run in bf16 on the PE (1 cycle/row vs fp32's 4) with fp32 PSUM
accumulation.  Per core the tokens are processed as two 512-token groups:
GEMM1 (64 rank-128 i-chunks) produces h^T tiles [128, 512] that stay resident
in SBUF, then GEMM2 accumulates over all 64 i-chunks into PSUM for 4 output
column chunks of 512.  Weights stream from DRAM twice (once per group), which
the DMA engines hide entirely under the PE's compute.

LayerNorm statistics use bn_stats/bn_aggr on the vector engine.  gamma/beta
are folded into W1/b1 host-side (ln@W1+b1 == z@(gamma*W1) + (beta@W1+b1)),
so the [tok,H] -> [H,tok] PE transposes drain PSUM->SBUF as plain copies,
four 128-column chunks per activation instruction.  GEMM1 starts on a
256-token sub-pass as soon as the first two token tiles' LayerNorm is done,
and GEMM2's PSUM accumulators rotate through six tag slots so column passes
never wait on drains.
"""

import sys

sys.path.insert(0, "/opt/trn_rl_repo")

import numpy as np

try:
    import jax

    jax.config.update("jax_compilation_cache_dir", "/tmp/jax_neff_cache")
    jax.config.update("jax_persistent_cache_min_compile_time_secs", 1.0)
    jax.config.update("jax_persistent_cache_min_entry_size_bytes", 0)
except Exception:
    pass

import ml_dtypes

import concourse.bass as bass  # noqa: F401
import concourse.mybir as mybir
from concourse import bacc
from concourse.masks import make_identity
from concourse.tile import TileContext

F32 = mybir.dt.float32
BF16 = mybir.dt.bfloat16
AF = mybir.ActivationFunctionType
ALU = mybir.AluOpType
NP_BF16 = ml_dtypes.bfloat16
U32 = mybir.dt.uint32

N_CORES = 8
B, S, H, I = 4, 2048, 2048, 8192
LN_EPS = 1e-6
NTOK = B * S                 # 8192 tokens total
TLOC = NTOK // N_CORES       # 1024 tokens per core
TT = TLOC // 128             # 8 token tiles per core
HC = H // 128                # 16 hidden chunks (contraction for GEMM1)
IC = I // 128                # 64 intermediate chunks
GROUPS = 2                   # token groups per core
GT = TT // GROUPS            # 4 token tiles per group
GTOK = TLOC // GROUPS        # 512 tokens per group
OC = H // 512                # 4 output column chunks of 512

_CACHE = {}


def _build_program():
    nc = bacc.Bacc("TRN2", target_bir_lowering=False, debug=False,
                   num_devices=N_CORES)

    xin = nc.declare_dram_parameter("xin", [TLOC, H], BF16, isOutput=False)
    xres = nc.declare_dram_parameter("xres", [TLOC, H], BF16, isOutput=False)
    # w1p[i, p, c*128 + f] = (gamma[:, None] * inter_w)[c*128 + p, i*128 + f]
    # (LayerNorm's gamma/beta are folded into W1/b1 host-side)
    w1p = nc.declare_dram_parameter("w1p", [IC, 128, H], BF16, isOutput=False)
    w2p = nc.declare_dram_parameter("w2p", [I, H], BF16, isOutput=False)
    bbt = nc.declare_dram_parameter("bbt", [128, H], BF16, isOutput=False)
    obt = nc.declare_dram_parameter("obt", [128, H], BF16, isOutput=False)
    # b1t[p, i] = (beta @ inter_w + inter_b)[i*128 + p]
    b1t = nc.declare_dram_parameter("b1t", [128, IC], F32, isOutput=False)
    out = nc.declare_dram_parameter("out", [TLOC, H], F32, isOutput=True)

    with TileContext(nc) as tc:
        with (
            tc.tile_pool(name="perm", bufs=1) as perm,
            tc.tile_pool(name="p1", bufs=2) as p1,
            tc.tile_pool(name="w1pool", bufs=4) as w1pool,
            tc.tile_pool(name="w2pool", bufs=6) as w2pool,
            tc.tile_pool(name="htpool", bufs=IC) as htpool,
            tc.tile_pool(name="osbp", bufs=4) as osbp,
            tc.tile_pool(name="ps", bufs=1, space="PSUM") as ps,
        ):
            ident = perm.tile([128, 128], BF16)
            eps = perm.tile([128, 1], F32)
            b1 = perm.tile([128, IC], F32)
            bb = perm.tile([128, H], BF16)
            ob = perm.tile([128, H], BF16)

            # ln^T, chunk-major: lnt[:, c, tok] = ln[tok, c*128 + p]
            lnta = perm.tile([128, HC, TLOC], BF16, name="lnta")
            lnt = lnta[:]
            rao = [perm.tile([128, H], BF16, name=f"rao{t}")
                   for t in range(TT)]

            zs = {}

            HH = H // 2

            def p1_load(t, split=False):
                tin = p1.tile([128, H], BF16, tag="tin")
                tre = p1.tile([128, H], BF16, tag="tre")
                rows = slice(t * 128, (t + 1) * 128)
                if split:
                    # half-tile loads alternating SP/ACT queues: compute can
                    # start after half the bytes have landed (the LayerNorm
                    # chain start gates the PE)
                    nc.sync.dma_start(out=tin[:, :HH], in_=xin[rows, :HH])
                    nc.scalar.dma_start(out=tre[:, :HH], in_=xres[rows, :HH])
                    nc.scalar.dma_start(out=tin[:, HH:], in_=xin[rows, HH:])
                    nc.sync.dma_start(out=tre[:, HH:], in_=xres[rows, HH:])
                else:
                    nc.sync.dma_start(out=tin[:], in_=xin[rows, :])
                    nc.sync.dma_start(out=tre[:], in_=xres[rows, :])
                return tin, tre

            def p1_compute(t, loaded=None, split=False):
                """residual add + LN stats + normalize for token tile t."""
                tin, tre = loaded if loaded is not None else p1_load(t)
                ra = p1.tile([128, H], BF16, tag="ra")
                if split:
                    for h in (slice(0, HH), slice(HH, H)):
                        nc.vector.tensor_add(ra[:, h], tin[:, h], tre[:, h])
                        nc.vector.tensor_add(ra[:, h], ra[:, h], bb[:, h])
                else:
                    nc.vector.tensor_add(ra[:], tin[:], tre[:])
                    nc.vector.tensor_add(ra[:], ra[:], bb[:])
                # final-residual term (ra + output_b), off the critical path
                nc.gpsimd.tensor_add(rao[t][:], ra[:], ob[:])
                # mean/var via bn_stats over 4 chunks of 512
                stats = p1.tile([128, 4, 6], F32, tag="stats")
                rav = ra[:].rearrange("p (n f) -> p n f", f=512)
                for sub in range(4):
                    nc.vector.bn_stats(stats[:, sub, :], rav[:, sub, :])
                mv = p1.tile([128, 2], F32, tag="mv")
                nc.vector.bn_aggr(mv[:], stats[:])
                std = p1.tile([128, 1], F32, tag="std")
                nc.scalar.activation(std[:], mv[:, 1:2], AF.Sqrt,
                                     bias=eps[:])
                rstd = p1.tile([128, 1], F32, tag="rstd")
                nc.vector.reciprocal(rstd[:], std[:])
                z = p1.tile([128, H], BF16, tag="z", bufs=4)
                nc.vector.tensor_scalar(
                    z[:], ra[:], mv[:, 0:1], rstd[:],
                    op0=ALU.subtract, op1=ALU.mult)
                zs[t] = z

            def p1_transpose(t):
                """z[tok, H] -> lnt[:, c, tok]; 4 chunks per PSUM drain."""
                z = zs[t]
                for cq in range(HC // 4):
                    tr = ps.tile([128, 512], BF16, tag="trp", bufs=2)
                    for j in range(4):
                        c = cq * 4 + j
                        nc.tensor.transpose(
                            tr[:, j * 128:(j + 1) * 128],
                            z[:, c * 128:(c + 1) * 128], ident[:])
                    trv = tr[:].rearrange("p (n f) -> p n f", f=128)
                    nc.scalar.activation(
                        lnt[:, cq * 4:(cq + 1) * 4, t * 128:(t + 1) * 128],
                        trv, AF.Copy)

            hts = [[None] * IC for _ in range(GROUPS)]

            def w1_load(i):
                w1t = w1pool.tile([128, H], BF16, tag="w1t")
                nc.sync.dma_start(out=w1t[:], in_=w1p[i])
                return w1t

            def g1_chunk(g, i, w1t=None, sub=None):
                """h^T[i-block] = gelu(W1^T @ ln^T + b1) for group g.

                sub=None computes all GTOK tokens; sub=0/1 computes the
                first/second 256-token half (used to start the PE before
                the later token tiles' LayerNorm has finished).
                """
                if w1t is None:
                    w1t = w1_load(i)
                if sub is None:
                    lo, n = 0, GTOK
                elif isinstance(sub, tuple):
                    lo, n = sub
                else:
                    lo, n = sub * (GTOK // 2), GTOK // 2
                psh = ps.tile([128, GTOK], F32, tag="psh", bufs=2)
                for c in range(HC):
                    nc.tensor.matmul(
                        psh[:, :n],
                        w1t[:, c * 128:(c + 1) * 128],
                        lnt[:, c, g * GTOK + lo:g * GTOK + lo + n],
                        start=(c == 0), stop=(c == HC - 1))
                if lo == 0:
                    ht = htpool.tile([128, GTOK], BF16, tag="ht")
                    hts[g][i] = ht
                nc.scalar.activation(hts[g][i][:, lo:lo + n], psh[:, :n],
                                     AF.Gelu, bias=b1[:, i:i + 1])

            def w2_load(oc, i):
                w2c = w2pool.tile([128, 512], BF16, tag="w2c")
                nc.scalar.dma_start(
                    out=w2c[:],
                    in_=w2p[i * 128:(i + 1) * 128, oc * 512:(oc + 1) * 512])
                return w2c

            # GEMM2 PSUM accumulators rotate through 6 tag slots (4 dedicated
            # + the GEMM1/transpose banks, idle during a GEMM2 pass) so a new
            # column pass never waits on the previous pass's drains.
            pso_slots = [("pso0", 1), ("pso1", 1), ("pso2", 1), ("pso3", 1),
                         ("psh", 2), ("trp", 2)]
            pso_cnt = [0]

            def g2_group(g, preloaded=()):
                """out[group tokens] = h @ W2 + (ra + output_b)."""
                for oc in range(OC):
                    psos = []
                    for t in range(GT):
                        tag, nb = pso_slots[(pso_cnt[0] + t) % len(pso_slots)]
                        psos.append(ps.tile([128, 512], F32,
                                            name=f"pso_{g}_{oc}_{t}",
                                            tag=tag, bufs=nb))
                    pso_cnt[0] += GT
                    last = (g == GROUPS - 1) and (oc == OC - 1)
                    o_lo = oc * 512

                    def drain(t, eng):
                        tt = g * GT + t
                        osb = osbp.tile([128, 512], F32, tag="osb",
                                        name="osb")
                        nc.vector.tensor_add(
                            osb[:], psos[t][:], rao[tt][:, o_lo:o_lo + 512])
                        eng.dma_start(
                            out=out[tt * 128:(tt + 1) * 128,
                                    o_lo:o_lo + 512],
                            in_=osb[:])

                    ntail = 4 if last else 0
                    for i in range(IC - ntail):
                        if oc == 0 and i < len(preloaded):
                            w2c = preloaded[i]
                        else:
                            w2c = w2_load(oc, i)
                        ht = hts[g][i]
                        for t in range(GT):
                            nc.tensor.matmul(
                                psos[t][:],
                                ht[:, t * 128:(t + 1) * 128],
                                w2c[:],
                                start=(i == 0), stop=(not last and i == IC - 1))
                    if not last:
                        for t in range(GT):
                            drain(t, nc.sync)
                    else:
                        # stagger the final stops tile-by-tile (the last few
                        # W2 tiles are still buffer-resident) so the output
                        # flush chains overlap the remaining matmuls
                        w2tail = {i: w2_load(oc, i)
                                  for i in range(IC - ntail, IC)}
                        for t in range(GT):
                            for i in range(IC - ntail, IC):
                                nc.tensor.matmul(
                                    psos[t][:],
                                    hts[g][i][:, t * 128:(t + 1) * 128],
                                    w2tail[i][:],
                                    start=False, stop=(i == IC - 1))
                            drain(t, nc.sync if t % 2 == 0 else nc.scalar)

            # ---- emission order: pipeline phase 1 under GEMM1 of group 0 ----
            # DMA order puts tile 0/1 activations first so the LayerNorm
            # chain (the critical path to the first matmul) starts ASAP.
            NSUB = 16   # leading GEMM1 i-chunks run as two 256-token passes
            l0 = p1_load(0, split=True)
            nc.sync.dma_start(out=bb[:, :HH], in_=bbt[:, :HH])
            nc.scalar.dma_start(out=bb[:, HH:], in_=bbt[:, HH:])
            l1 = p1_load(1, split=True)
            nc.sync.dma_start(out=ob[:], in_=obt[:])
            nc.sync.dma_start(out=b1[:], in_=b1t[:])
            make_identity(nc, ident[:])
            nc.vector.memset(eps[:], LN_EPS)
            with tc.high_priority():
                p1_compute(0, l0, split=True)
                p1_compute(1, l1, split=True)
                p1_transpose(0)
                p1_transpose(1)
            p1_compute(2)
            p1_compute(3)

            # first NSUB chunks: tokens 0-255 only (needs just tiles 0-1), so
            # the PE starts as soon as the first two LayerNorm tiles are done
            w1_first = [w1_load(i) for i in range(min(3, NSUB))]
            for i in range(0, NSUB):
                g1_chunk(0, i, w1t=w1_first[i] if i < len(w1_first) else None,
                         sub=0)
            # prefetch the re-loads for the second 256-token pass
            w1_sub1 = [w1_load(i) for i in range(2)]
            p1_transpose(2)
            p1_transpose(3)
            p1_compute(4)
            # second half of the leading chunks (tokens 256-511)
            for i in range(0, NSUB):
                g1_chunk(0, i, w1t=w1_sub1[i] if i < len(w1_sub1) else None,
                         sub=1)
            p1_compute(5)
            for i in range(NSUB, 16):
                g1_chunk(0, i)
            p1_transpose(4)
            p1_compute(6)
            for i in range(16, 24):
                g1_chunk(0, i)
            p1_transpose(5)
            p1_compute(7)
            for i in range(24, 32):
                g1_chunk(0, i)
            p1_transpose(6)
            for i in range(32, 40):
                g1_chunk(0, i)
            p1_transpose(7)
            for i in range(40, IC - 8):
                g1_chunk(0, i)
            # prefetch the first W2 column chunks (ACT queue) so GEMM2 starts
            # seamlessly after GEMM1's last chunk
            w2_first = [w2_load(0, i) for i in range(4)]
            for i in range(IC - 8, IC):
                g1_chunk(0, i)

            g2_group(0, preloaded=w2_first)
            for i in range(IC - 8):
                g1_chunk(1, i)
            w2_g1 = [w2_load(0, i) for i in range(4)]
            for i in range(IC - 8, IC):
                g1_chunk(1, i)
            g2_group(1, preloaded=w2_g1)

    nc.compile()
    return nc


def _get_program():
    if "nc" not in _CACHE:
        _CACHE["nc"] = _build_program()
    return _CACHE["nc"]


def kernel(input, residual, residual_norm, bias, gamma, beta,
           inter_w, inter_b, output_w, output_b):
    nc = _get_program()

    input = np.asarray(input, dtype=np.float32)
    residual = np.asarray(residual, dtype=np.float32)
    bias = np.asarray(bias, dtype=np.float32)
    gamma = np.asarray(gamma, dtype=np.float32)
    beta = np.asarray(beta, dtype=np.float32)
    inter_w = np.asarray(inter_w, dtype=np.float32)
    inter_b = np.asarray(inter_b, dtype=np.float32)
    output_w = np.asarray(output_w, dtype=np.float32)
    output_b = np.asarray(output_b, dtype=np.float32)

    xin = np.ascontiguousarray(input.reshape(NTOK, H).astype(NP_BF16))
    xres = np.ascontiguousarray(residual.reshape(NTOK, H).astype(NP_BF16))
    # fold LayerNorm's gamma/beta into W1/b1:
    #   ln @ W1 + b1 == z @ (gamma[:,None]*W1) + (beta @ W1 + b1)
    w1f = gamma[:, None].astype(np.float32) * inter_w
    b1f = beta.astype(np.float32) @ inter_w + inter_b
    # w1p[i, p, c*128+f] = w1f[c*128+p, i*128+f]
    w1p = np.ascontiguousarray(
        w1f.reshape(HC, 128, IC, 128).transpose(2, 1, 0, 3)
        .reshape(IC, 128, H).astype(NP_BF16))
    w2p = np.ascontiguousarray(output_w.astype(NP_BF16))
    bbt = np.ascontiguousarray(
        np.broadcast_to(bias.astype(NP_BF16), (128, H)))
    obt = np.ascontiguousarray(
        np.broadcast_to(output_b.astype(NP_BF16), (128, H)))
    b1t = np.ascontiguousarray(b1f.reshape(IC, 128).T)

    in_maps = []
    for c in range(N_CORES):
        in_maps.append({
            "xin": np.ascontiguousarray(xin[c * TLOC:(c + 1) * TLOC]),
            "xres": np.ascontiguousarray(xres[c * TLOC:(c + 1) * TLOC]),
            "w1p": w1p,
            "w2p": w2p,
            "bbt": bbt,
            "obt": obt,
            "b1t": b1t,
        })

    from concourse.bass_utils import run_bass_kernel_spmd
    res = run_bass_kernel_spmd(nc, in_maps, list(range(N_CORES)))
    out = np.concatenate([res.results[c]["out"] for c in range(N_CORES)],
                         axis=0)
    return out.reshape(B, S, H)


if __name__ == "__main__":
    nc = _get_program()
    from concourse.timeline_sim import TimelineSim
    ts = TimelineSim(nc)
    total = ts.simulate()
    print(f"TimelineSim: {total:.0f} ns")


# revision 57
# speedup vs baseline: 1.0016x; 1.0001x over previous
"""DeepSpeed-style MLP block (residual-add + LayerNorm + GEMM + GeLU + GEMM +
residual) on 8 Trainium2 NeuronCores.

Sharding: data-parallel over tokens (B*S = 8192 -> 1024 tokens/core).  Each
core holds the full weights and computes its token slice end-to-end; no
collectives.

All matmuls run in bf16 on the PE (1 cycle/row vs fp32's 4) with fp32 PSUM
accumulation.  Per core the tokens are processed as two 512-token groups:
GEMM1 (64 rank-128 i-chunks) produces h^T tiles [128, 512] that stay resident
in SBUF, then GEMM2 accumulates over all 64 i-chunks into PSUM for 4 output
column chunks of 512.  Weights stream from DRAM twice (once per group), which
the DMA engines hide entirely under the PE's compute.

LayerNorm statistics use bn_stats/bn_aggr on the vector engine.  gamma/beta
are folded into W1/b1 host-side (ln@W1+b1 == z@(gamma*W1) + (beta@W1+b1)),
so the [tok,H] -> [H,tok] PE transposes drain PSUM->SBUF as plain copies,
four 128-column chunks per activation instruction.  GEMM1 starts on a
256-token sub-pass as soon as the first two token tiles' LayerNorm is done,
and GEMM2's PSUM accumulators rotate through six tag slots so column passes
never wait on drains.
"""

import sys

sys.path.insert(0, "/opt/trn_rl_repo")

import numpy as np

try:
    import jax

    jax.config.update("jax_compilation_cache_dir", "/tmp/jax_neff_cache")
    jax.config.update("jax_persistent_cache_min_compile_time_secs", 1.0)
    jax.config.update("jax_persistent_cache_min_entry_size_bytes", 0)
except Exception:
    pass

import ml_dtypes

import concourse.bass as bass  # noqa: F401
import concourse.mybir as mybir
from concourse import bacc
from concourse.masks import make_identity
from concourse.tile import TileContext

F32 = mybir.dt.float32
BF16 = mybir.dt.bfloat16
AF = mybir.ActivationFunctionType
ALU = mybir.AluOpType
NP_BF16 = ml_dtypes.bfloat16
U32 = mybir.dt.uint32

N_CORES = 8
B, S, H, I = 4, 2048, 2048, 8192
LN_EPS = 1e-6
NTOK = B * S                 # 8192 tokens total
TLOC = NTOK // N_CORES       # 1024 tokens per core
TT = TLOC // 128             # 8 token tiles per core
HC = H // 128                # 16 hidden chunks (contraction for GEMM1)
IC = I // 128                # 64 intermediate chunks
GROUPS = 2                   # token groups per core
GT = TT // GROUPS            # 4 token tiles per group
GTOK = TLOC // GROUPS        # 512 tokens per group
OC = H // 512                # 4 output column chunks of 512

_CACHE = {}


def _build_program():
    nc = bacc.Bacc("TRN2", target_bir_lowering=False, debug=False,
                   num_devices=N_CORES)

    xin = nc.declare_dram_parameter("xin", [TLOC, H], BF16, isOutput=False)
    xres = nc.declare_dram_parameter("xres", [TLOC, H], BF16, isOutput=False)
    # w1p[i, p, c*128 + f] = (gamma[:, None] * inter_w)[c*128 + p, i*128 + f]
    # (LayerNorm's gamma/beta are folded into W1/b1 host-side)
    w1p = nc.declare_dram_parameter("w1p", [IC, 128, H], BF16, isOutput=False)
    w2p = nc.declare_dram_parameter("w2p", [I, H], BF16, isOutput=False)
    bbt = nc.declare_dram_parameter("bbt", [128, H], BF16, isOutput=False)
    obt = nc.declare_dram_parameter("obt", [128, H], BF16, isOutput=False)
    # b1t[p, i] = (beta @ inter_w + inter_b)[i*128 + p]
    b1t = nc.declare_dram_parameter("b1t", [128, IC], F32, isOutput=False)
    out = nc.declare_dram_parameter("out", [TLOC, H], F32, isOutput=True)

    with TileContext(nc) as tc:
        with (
            tc.tile_pool(name="perm", bufs=1) as perm,
            tc.tile_pool(name="p1", bufs=2) as p1,
            tc.tile_pool(name="w1pool", bufs=4) as w1pool,
            tc.tile_pool(name="w2pool", bufs=6) as w2pool,
            tc.tile_pool(name="htpool", bufs=IC) as htpool,
            tc.tile_pool(name="osbp", bufs=4) as osbp,
            tc.tile_pool(name="ps", bufs=1, space="PSUM") as ps,
        ):
            ident = perm.tile([128, 128], BF16)
            eps = perm.tile([128, 1], F32)
            b1 = perm.tile([128, IC], F32)
            bb = perm.tile([128, H], BF16)
            ob = perm.tile([128, H], BF16)

            # ln^T, chunk-major: lnt[:, c, tok] = ln[tok, c*128 + p]
            lnta = perm.tile([128, HC, TLOC], BF16, name="lnta")
            lnt = lnta[:]
            rao = [perm.tile([128, H], BF16, name=f"rao{t}")
                   for t in range(TT)]

            zs = {}

            HH = H // 2

            def p1_load(t, split=False):
                tin = p1.tile([128, H], BF16, tag="tin")
                tre = p1.tile([128, H], BF16, tag="tre")
                rows = slice(t * 128, (t + 1) * 128)
                if split:
                    # half-tile loads alternating SP/ACT queues: compute can
                    # start after half the bytes have landed (the LayerNorm
                    # chain start gates the PE)
                    nc.sync.dma_start(out=tin[:, :HH], in_=xin[rows, :HH])
                    nc.scalar.dma_start(out=tre[:, :HH], in_=xres[rows, :HH])
                    nc.scalar.dma_start(out=tin[:, HH:], in_=xin[rows, HH:])
                    nc.sync.dma_start(out=tre[:, HH:], in_=xres[rows, HH:])
                else:
                    nc.sync.dma_start(out=tin[:], in_=xin[rows, :])
                    nc.sync.dma_start(out=tre[:], in_=xres[rows, :])
                return tin, tre

            def p1_compute(t, loaded=None, split=False):
                """residual add + LN stats + normalize for token tile t."""
                tin, tre = loaded if loaded is not None else p1_load(t)
                ra = p1.tile([128, H], BF16, tag="ra")
                if split:
                    for h in (slice(0, HH), slice(HH, H)):
                        nc.vector.tensor_add(ra[:, h], tin[:, h], tre[:, h])
                        nc.vector.tensor_add(ra[:, h], ra[:, h], bb[:, h])
                else:
                    nc.vector.tensor_add(ra[:], tin[:], tre[:])
                    nc.vector.tensor_add(ra[:], ra[:], bb[:])
                # final-residual term (ra + output_b), off the critical path
                nc.gpsimd.tensor_add(rao[t][:], ra[:], ob[:])
                # mean/var via bn_stats over 4 chunks of 512
                stats = p1.tile([128, 4, 6], F32, tag="stats")
                rav = ra[:].rearrange("p (n f) -> p n f", f=512)
                for sub in range(4):
                    nc.vector.bn_stats(stats[:, sub, :], rav[:, sub, :])
                mv = p1.tile([128, 2], F32, tag="mv")
                nc.vector.bn_aggr(mv[:], stats[:])
                std = p1.tile([128, 1], F32, tag="std")
                nc.scalar.activation(std[:], mv[:, 1:2], AF.Sqrt,
                                     bias=eps[:])
                rstd = p1.tile([128, 1], F32, tag="rstd")
                nc.vector.reciprocal(rstd[:], std[:])
                z = p1.tile([128, H], BF16, tag="z", bufs=4)
                nc.vector.tensor_scalar(
                    z[:], ra[:], mv[:, 0:1], rstd[:],
                    op0=ALU.subtract, op1=ALU.mult)
                zs[t] = z

            def p1_transpose(t):
                """z[tok, H] -> lnt[:, c, tok]; 4 chunks per PSUM drain."""
                z = zs[t]
                for cq in range(HC // 4):
                    tr = ps.tile([128, 512], BF16, tag="trp", bufs=2)
                    for j in range(4):
                        c = cq * 4 + j
                        nc.tensor.transpose(
                            tr[:, j * 128:(j + 1) * 128],
                            z[:, c * 128:(c + 1) * 128], ident[:])
                    trv = tr[:].rearrange("p (n f) -> p n f", f=128)
                    nc.scalar.activation(
                        lnt[:, cq * 4:(cq + 1) * 4, t * 128:(t + 1) * 128],
                        trv, AF.Copy)

            hts = [[None] * IC for _ in range(GROUPS)]

            def w1_load(i):
                w1t = w1pool.tile([128, H], BF16, tag="w1t")
                nc.sync.dma_start(out=w1t[:], in_=w1p[i])
                return w1t

            def g1_chunk(g, i, w1t=None, sub=None):
                """h^T[i-block] = gelu(W1^T @ ln^T + b1) for group g.

                sub=None computes all GTOK tokens; sub=0/1 computes the
                first/second 256-token half (used to start the PE before
                the later token tiles' LayerNorm has finished).
                """
                if w1t is None:
                    w1t = w1_load(i)
                if sub is None:
                    lo, n = 0, GTOK
                elif isinstance(sub, tuple):
                    lo, n = sub
                else:
                    lo, n = sub * (GTOK // 2), GTOK // 2
                psh = ps.tile([128, GTOK], F32, tag="psh", bufs=2)
                for c in range(HC):
                    nc.tensor.matmul(
                        psh[:, :n],
                        w1t[:, c * 128:(c + 1) * 128],
                        lnt[:, c, g * GTOK + lo:g * GTOK + lo + n],
                        start=(c == 0), stop=(c == HC - 1))
                if lo == 0:
                    ht = htpool.tile([128, GTOK], BF16, tag="ht")
                    hts[g][i] = ht
                nc.scalar.activation(hts[g][i][:, lo:lo + n], psh[:, :n],
                                     AF.Gelu, bias=b1[:, i:i + 1])

            def w2_load(oc, i):
                w2c = w2pool.tile([128, 512], BF16, tag="w2c")
                nc.scalar.dma_start(
                    out=w2c[:],
                    in_=w2p[i * 128:(i + 1) * 128, oc * 512:(oc + 1) * 512])
                return w2c

            # GEMM2 PSUM accumulators rotate through 6 tag slots (4 dedicated
            # + the GEMM1/transpose banks, idle during a GEMM2 pass) so a new
            # column pass never waits on the previous pass's drains.
            pso_slots = [("pso0", 1), ("pso1", 1), ("pso2", 1), ("pso3", 1),
                         ("psh", 2), ("trp", 2)]
            pso_cnt = [0]

            def g2_group(g, preloaded=()):
                """out[group tokens] = h @ W2 + (ra + output_b)."""
                for oc in range(OC):
                    psos = []
                    for t in range(GT):
                        tag, nb = pso_slots[(pso_cnt[0] + t) % len(pso_slots)]
                        psos.append(ps.tile([128, 512], F32,
                                            name=f"pso_{g}_{oc}_{t}",
                                            tag=tag, bufs=nb))
                    pso_cnt[0] += GT
                    last = (g == GROUPS - 1) and (oc == OC - 1)
                    o_lo = oc * 512

                    def drain(t, eng):
                        tt = g * GT + t
                        osb = osbp.tile([128, 512], F32, tag="osb",
                                        name="osb")
                        nc.vector.tensor_add(
                            osb[:], psos[t][:], rao[tt][:, o_lo:o_lo + 512])
                        eng.dma_start(
                            out=out[tt * 128:(tt + 1) * 128,
                                    o_lo:o_lo + 512],
                            in_=osb[:])

                    ntail = 4 if last else 0
                    for i in range(IC - ntail):
                        if oc == 0 and i < len(preloaded):
                            w2c = preloaded[i]
                        else:
                            w2c = w2_load(oc, i)
                        ht = hts[g][i]
                        for t in range(GT):
                            nc.tensor.matmul(
                                psos[t][:],
                                ht[:, t * 128:(t + 1) * 128],
                                w2c[:],
                                start=(i == 0), stop=(not last and i == IC - 1))
                    if not last:
                        for t in range(GT):
                            drain(t, nc.sync)
                    else:
                        # stagger the final stops tile-by-tile (the last few
                        # W2 tiles are still buffer-resident) so the output
                        # flush chains overlap the remaining matmuls
                        w2tail = {i: w2_load(oc, i)
                                  for i in range(IC - ntail, IC)}
                        for t in range(GT):
                            for i in range(IC - ntail, IC):
                                nc.tensor.matmul(
                                    psos[t][:],
                                    hts[g][i][:, t * 128:(t + 1) * 128],
                                    w2tail[i][:],
                                    start=False, stop=(i == IC - 1))
                            drain(t, nc.sync if t % 2 == 0 else nc.scalar)

            # ---- emission order: pipeline phase 1 under GEMM1 of group 0 ----
            # DMA order puts tile 0/1 activations first so the LayerNorm
            # chain (the critical path to the first matmul) starts ASAP.
            NSUB = 16   # leading GEMM1 i-chunks run as two 256-token passes
            l0 = p1_load(0, split=True)
            nc.sync.dma_start(out=bb[:, :HH], in_=bbt[:, :HH])
            nc.scalar.dma_start(out=bb[:, HH:], in_=bbt[:, HH:])
            l1 = p1_load(1, split=True)
            nc.sync.dma_start(out=ob[:], in_=obt[:])
            nc.sync.dma_start(out=b1[:], in_=b1t[:])
            make_identity(nc, ident[:])
            nc.vector.memset(eps[:], LN_EPS)
            with tc.high_priority():
                p1_compute(0, l0, split=True)
                p1_compute(1, l1)
                p1_transpose(0)
                p1_transpose(1)
            p1_compute(2)
            p1_compute(3)

            # first NSUB chunks: tokens 0-255 only (needs just tiles 0-1), so
            # the PE starts as soon as the first two LayerNorm tiles are done
            w1_first = [w1_load(i) for i in range(min(3, NSUB))]
            for i in range(0, NSUB):
                g1_chunk(0, i, w1t=w1_first[i] if i < len(w1_first) else None,
                         sub=0)
            # prefetch the re-loads for the second 256-token pass
            w1_sub1 = [w1_load(i) for i in range(2)]
            p1_transpose(2)
            p1_transpose(3)
            p1_compute(4)
            # second half of the leading chunks (tokens 256-511)
            for i in range(0, NSUB):
                g1_chunk(0, i, w1t=w1_sub1[i] if i < len(w1_sub1) else None,
                         sub=1)
            p1_compute(5)
            for i in range(NSUB, 16):
                g1_chunk(0, i)
            p1_transpose(4)
            p1_compute(6)
            for i in range(16, 24):
                g1_chunk(0, i)
            p1_transpose(5)
            p1_compute(7)
            for i in range(24, 32):
                g1_chunk(0, i)
            p1_transpose(6)
            for i in range(32, 40):
                g1_chunk(0, i)
            p1_transpose(7)
            for i in range(40, IC - 8):
                g1_chunk(0, i)
            # prefetch the first W2 column chunks (ACT queue) so GEMM2 starts
            # seamlessly after GEMM1's last chunk
            w2_first = [w2_load(0, i) for i in range(4)]
            for i in range(IC - 8, IC):
                g1_chunk(0, i)

            g2_group(0, preloaded=w2_first)
            for i in range(IC - 8):
                g1_chunk(1, i)
            w2_g1 = [w2_load(0, i) for i in range(4)]
            for i in range(IC - 8, IC):
                g1_chunk(1, i)
            g2_group(1, preloaded=w2_g1)

    nc.compile()
    return nc


def _get_program():
    if "nc" not in _CACHE:
        _CACHE["nc"] = _build_program()
    return _CACHE["nc"]


def kernel(input, residual, residual_norm, bias, gamma, beta,
           inter_w, inter_b, output_w, output_b):
    nc = _get_program()

    input = np.asarray(input, dtype=np.float32)
    residual = np.asarray(residual, dtype=np.float32)
    bias = np.asarray(bias, dtype=np.float32)
    gamma = np.asarray(gamma, dtype=np.float32)
    beta = np.asarray(beta, dtype=np.float32)
    inter_w = np.asarray(inter_w, dtype=np.float32)
    inter_b = np.asarray(inter_b, dtype=np.float32)
    output_w = np.asarray(output_w, dtype=np.float32)
    output_b = np.asarray(output_b, dtype=np.float32)

    xin = np.ascontiguousarray(input.reshape(NTOK, H).astype(NP_BF16))
    xres = np.ascontiguousarray(residual.reshape(NTOK, H).astype(NP_BF16))
    # fold LayerNorm's gamma/beta into W1/b1:
    #   ln @ W1 + b1 == z @ (gamma[:,None]*W1) + (beta @ W1 + b1)
    w1f = gamma[:, None].astype(np.float32) * inter_w
    b1f = beta.astype(np.float32) @ inter_w + inter_b
    # w1p[i, p, c*128+f] = w1f[c*128+p, i*128+f]
    w1p = np.ascontiguousarray(
        w1f.reshape(HC, 128, IC, 128).transpose(2, 1, 0, 3)
        .reshape(IC, 128, H).astype(NP_BF16))
    w2p = np.ascontiguousarray(output_w.astype(NP_BF16))
    bbt = np.ascontiguousarray(
        np.broadcast_to(bias.astype(NP_BF16), (128, H)))
    obt = np.ascontiguousarray(
        np.broadcast_to(output_b.astype(NP_BF16), (128, H)))
    b1t = np.ascontiguousarray(b1f.reshape(IC, 128).T)

    in_maps = []
    for c in range(N_CORES):
        in_maps.append({
            "xin": np.ascontiguousarray(xin[c * TLOC:(c + 1) * TLOC]),
            "xres": np.ascontiguousarray(xres[c * TLOC:(c + 1) * TLOC]),
            "w1p": w1p,
            "w2p": w2p,
            "bbt": bbt,
            "obt": obt,
            "b1t": b1t,
        })

    from concourse.bass_utils import run_bass_kernel_spmd
    res = run_bass_kernel_spmd(nc, in_maps, list(range(N_CORES)))
    out = np.concatenate([res.results[c]["out"] for c in range(N_CORES)],
                         axis=0)
    return out.reshape(B, S, H)


if __name__ == "__main__":
    nc = _get_program()
    from concourse.timeline_sim import TimelineSim
    ts = TimelineSim(nc)
    total = ts.simulate()
    print(f"TimelineSim: {total:.0f} ns")


# revision 63
# speedup vs baseline: 1.0016x; 1.0000x over previous
"""DeepSpeed-style MLP block (residual-add + LayerNorm + GEMM + GeLU + GEMM +
residual) on 8 Trainium2 NeuronCores.

Sharding: data-parallel over tokens (B*S = 8192 -> 1024 tokens/core).  Each
core holds the full weights and computes its token slice end-to-end; no
collectives.

All matmuls run in bf16 on the PE (1 cycle/row vs fp32's 4) with fp32 PSUM
accumulation.  Per core the tokens are processed as two 512-token groups:
GEMM1 (64 rank-128 i-chunks) produces h^T tiles [128, 512] that stay resident
in SBUF, then GEMM2 accumulates over all 64 i-chunks into PSUM for 4 output
column chunks of 512.  Weights stream from DRAM twice (once per group), which
the DMA engines hide entirely under the PE's compute.

LayerNorm statistics use bn_stats/bn_aggr on the vector engine.  gamma/beta
are folded into W1/b1 host-side (ln@W1+b1 == z@(gamma*W1) + (beta@W1+b1)),
so the [tok,H] -> [H,tok] PE transposes drain PSUM->SBUF as plain copies,
four 128-column chunks per activation instruction.  GEMM1 starts on a
256-token sub-pass as soon as the first two token tiles' LayerNorm is done,
and GEMM2's PSUM accumulators rotate through six tag slots so column passes
never wait on drains.
"""

import sys

sys.path.insert(0, "/opt/trn_rl_repo")

import numpy as np

try:
    import jax

    jax.config.update("jax_compilation_cache_dir", "/tmp/jax_neff_cache")
    jax.config.update("jax_persistent_cache_min_compile_time_secs", 1.0)
    jax.config.update("jax_persistent_cache_min_entry_size_bytes", 0)
except Exception:
    pass

import ml_dtypes

import concourse.bass as bass  # noqa: F401
import concourse.mybir as mybir
from concourse import bacc
from concourse.masks import make_identity
from concourse.tile import TileContext

F32 = mybir.dt.float32
BF16 = mybir.dt.bfloat16
AF = mybir.ActivationFunctionType
ALU = mybir.AluOpType
NP_BF16 = ml_dtypes.bfloat16
U32 = mybir.dt.uint32

N_CORES = 8
B, S, H, I = 4, 2048, 2048, 8192
LN_EPS = 1e-6
NTOK = B * S                 # 8192 tokens total
TLOC = NTOK // N_CORES       # 1024 tokens per core
TT = TLOC // 128             # 8 token tiles per core
HC = H // 128                # 16 hidden chunks (contraction for GEMM1)
IC = I // 128                # 64 intermediate chunks
GROUPS = 2                   # token groups per core
GT = TT // GROUPS            # 4 token tiles per group
GTOK = TLOC // GROUPS        # 512 tokens per group
OC = H // 512                # 4 output column chunks of 512

_CACHE = {}


def _build_program():
    nc = bacc.Bacc("TRN2", target_bir_lowering=False, debug=False,
                   num_devices=N_CORES)

    xin = nc.declare_dram_parameter("xin", [TLOC, H], BF16, isOutput=False)
    xres = nc.declare_dram_parameter("xres", [TLOC, H], BF16, isOutput=False)
    # w1p[i, p, c*128 + f] = (gamma[:, None] * inter_w)[c*128 + p, i*128 + f]
    # (LayerNorm's gamma/beta are folded into W1/b1 host-side)
    w1p = nc.declare_dram_parameter("w1p", [IC, 128, H], BF16, isOutput=False)
    w2p = nc.declare_dram_parameter("w2p", [I, H], BF16, isOutput=False)
    bbt = nc.declare_dram_parameter("bbt", [128, H], BF16, isOutput=False)
    obt = nc.declare_dram_parameter("obt", [128, H], BF16, isOutput=False)
    # b1t[p, i] = (beta @ inter_w + inter_b)[i*128 + p]
    b1t = nc.declare_dram_parameter("b1t", [128, IC], F32, isOutput=False)
    out = nc.declare_dram_parameter("out", [TLOC, H], F32, isOutput=True)

    with TileContext(nc) as tc:
        with (
            tc.tile_pool(name="perm", bufs=1) as perm,
            tc.tile_pool(name="p1", bufs=2) as p1,
            tc.tile_pool(name="w1pool", bufs=4) as w1pool,
            tc.tile_pool(name="w2pool", bufs=6) as w2pool,
            tc.tile_pool(name="htpool", bufs=IC) as htpool,
            tc.tile_pool(name="osbp", bufs=4) as osbp,
            tc.tile_pool(name="ps", bufs=1, space="PSUM") as ps,
        ):
            ident = perm.tile([128, 128], BF16)
            eps = perm.tile([128, 1], F32)
            b1 = perm.tile([128, IC], F32)
            bb = perm.tile([128, H], BF16)
            ob = perm.tile([128, H], BF16)

            # ln^T, chunk-major: lnt[:, c, tok] = ln[tok, c*128 + p]
            lnta = perm.tile([128, HC, TLOC], BF16, name="lnta")
            lnt = lnta[:]
            rao = [perm.tile([128, H], BF16, name=f"rao{t}")
                   for t in range(TT)]

            zs = {}

            HH = H // 2

            def p1_load(t, split=False):
                tin = p1.tile([128, H], BF16, tag="tin")
                tre = p1.tile([128, H], BF16, tag="tre")
                rows = slice(t * 128, (t + 1) * 128)
                if split:
                    # half-tile loads alternating SP/ACT queues: compute can
                    # start after half the bytes have landed (the LayerNorm
                    # chain start gates the PE)
                    nc.sync.dma_start(out=tin[:, :HH], in_=xin[rows, :HH])
                    nc.scalar.dma_start(out=tre[:, :HH], in_=xres[rows, :HH])
                    nc.scalar.dma_start(out=tin[:, HH:], in_=xin[rows, HH:])
                    nc.sync.dma_start(out=tre[:, HH:], in_=xres[rows, HH:])
                else:
                    nc.sync.dma_start(out=tin[:], in_=xin[rows, :])
                    nc.sync.dma_start(out=tre[:], in_=xres[rows, :])
                return tin, tre

            def p1_compute(t, loaded=None, split=False):
                """residual add + LN stats + normalize for token tile t."""
                tin, tre = loaded if loaded is not None else p1_load(t)
                ra = p1.tile([128, H], BF16, tag="ra")
                if split:
                    for h in (slice(0, HH), slice(HH, H)):
                        nc.vector.tensor_add(ra[:, h], tin[:, h], tre[:, h])
                        nc.vector.tensor_add(ra[:, h], ra[:, h], bb[:, h])
                else:
                    nc.vector.tensor_add(ra[:], tin[:], tre[:])
                    nc.vector.tensor_add(ra[:], ra[:], bb[:])
                # final-residual term (ra + output_b), off the critical path
                nc.gpsimd.tensor_add(rao[t][:], ra[:], ob[:])
                # mean/var via bn_stats over 4 chunks of 512
                stats = p1.tile([128, 4, 6], F32, tag="stats")
                rav = ra[:].rearrange("p (n f) -> p n f", f=512)
                for sub in range(4):
                    nc.vector.bn_stats(stats[:, sub, :], rav[:, sub, :])
                mv = p1.tile([128, 2], F32, tag="mv")
                nc.vector.bn_aggr(mv[:], stats[:])
                std = p1.tile([128, 1], F32, tag="std")
                nc.scalar.activation(std[:], mv[:, 1:2], AF.Sqrt,
                                     bias=eps[:])
                rstd = p1.tile([128, 1], F32, tag="rstd")
                nc.vector.reciprocal(rstd[:], std[:])
                z = p1.tile([128, H], BF16, tag="z", bufs=4)
                nc.vector.tensor_scalar(
                    z[:], ra[:], mv[:, 0:1], rstd[:],
                    op0=ALU.subtract, op1=ALU.mult)
                zs[t] = z

            def p1_transpose(t):
                """z[tok, H] -> lnt[:, c, tok]; 4 chunks per PSUM drain."""
                z = zs[t]
                for cq in range(HC // 4):
                    tr = ps.tile([128, 512], BF16, tag="trp", bufs=2)
                    for j in range(4):
                        c = cq * 4 + j
                        nc.tensor.transpose(
                            tr[:, j * 128:(j + 1) * 128],
                            z[:, c * 128:(c + 1) * 128], ident[:])
                    trv = tr[:].rearrange("p (n f) -> p n f", f=128)
                    nc.scalar.activation(
                        lnt[:, cq * 4:(cq + 1) * 4, t * 128:(t + 1) * 128],
                        trv, AF.Copy)

            def p1_transpose_dma(t):
                """Same as p1_transpose but via the DMA XBAR (no PE/ACT
                work).  Emit only where z[t] is certainly ready: the issue
                blocks the SP queue until then."""
                nc.sync.dma_start_transpose(
                    out=lnt[:, :, t * 128:(t + 1) * 128], in_=zs[t][:])

            hts = [[None] * IC for _ in range(GROUPS)]

            def w1_load(i):
                w1t = w1pool.tile([128, H], BF16, tag="w1t")
                nc.sync.dma_start(out=w1t[:], in_=w1p[i])
                return w1t

            def g1_chunk(g, i, w1t=None, sub=None):
                """h^T[i-block] = gelu(W1^T @ ln^T + b1) for group g.

                sub=None computes all GTOK tokens; sub=0/1 computes the
                first/second 256-token half (used to start the PE before
                the later token tiles' LayerNorm has finished).
                """
                if w1t is None:
                    w1t = w1_load(i)
                if sub is None:
                    lo, n = 0, GTOK
                elif isinstance(sub, tuple):
                    lo, n = sub
                else:
                    lo, n = sub * (GTOK // 2), GTOK // 2
                psh = ps.tile([128, GTOK], F32, tag="psh", bufs=2)
                for c in range(HC):
                    nc.tensor.matmul(
                        psh[:, :n],
                        w1t[:, c * 128:(c + 1) * 128],
                        lnt[:, c, g * GTOK + lo:g * GTOK + lo + n],
                        start=(c == 0), stop=(c == HC - 1))
                if lo == 0:
                    ht = htpool.tile([128, GTOK], BF16, tag="ht")
                    hts[g][i] = ht
                nc.scalar.activation(hts[g][i][:, lo:lo + n], psh[:, :n],
                                     AF.Gelu, bias=b1[:, i:i + 1])

            def w2_load(oc, i):
                w2c = w2pool.tile([128, 512], BF16, tag="w2c")
                nc.scalar.dma_start(
                    out=w2c[:],
                    in_=w2p[i * 128:(i + 1) * 128, oc * 512:(oc + 1) * 512])
                return w2c

            # GEMM2 PSUM accumulators rotate through 6 tag slots (4 dedicated
            # + the GEMM1/transpose banks, idle during a GEMM2 pass) so a new
            # column pass never waits on the previous pass's drains.
            pso_slots = [("pso0", 1), ("pso1", 1), ("pso2", 1), ("pso3", 1),
                         ("psh", 2), ("trp", 2)]
            pso_cnt = [0]

            def g2_group(g, preloaded=()):
                """out[group tokens] = h @ W2 + (ra + output_b)."""
                for oc in range(OC):
                    psos = []
                    for t in range(GT):
                        tag, nb = pso_slots[(pso_cnt[0] + t) % len(pso_slots)]
                        psos.append(ps.tile([128, 512], F32,
                                            name=f"pso_{g}_{oc}_{t}",
                                            tag=tag, bufs=nb))
                    pso_cnt[0] += GT
                    last = (g == GROUPS - 1) and (oc == OC - 1)
                    o_lo = oc * 512

                    def drain(t, eng):
                        tt = g * GT + t
                        osb = osbp.tile([128, 512], F32, tag="osb",
                                        name="osb")
                        nc.vector.tensor_add(
                            osb[:], psos[t][:], rao[tt][:, o_lo:o_lo + 512])
                        eng.dma_start(
                            out=out[tt * 128:(tt + 1) * 128,
                                    o_lo:o_lo + 512],
                            in_=osb[:])

                    ntail = 4 if last else 0
                    for i in range(IC - ntail):
                        if oc == 0 and i < len(preloaded):
                            w2c = preloaded[i]
                        else:
                            w2c = w2_load(oc, i)
                        ht = hts[g][i]
                        for t in range(GT):
                            nc.tensor.matmul(
                                psos[t][:],
                                ht[:, t * 128:(t + 1) * 128],
                                w2c[:],
                                start=(i == 0), stop=(not last and i == IC - 1))
                    if not last:
                        for t in range(GT):
                            drain(t, nc.sync)
                    else:
                        # stagger the final stops tile-by-tile (the last few
                        # W2 tiles are still buffer-resident) so the output
                        # flush chains overlap the remaining matmuls
                        w2tail = {i: w2_load(oc, i)
                                  for i in range(IC - ntail, IC)}
                        for t in range(GT):
                            for i in range(IC - ntail, IC):
                                nc.tensor.matmul(
                                    psos[t][:],
                                    hts[g][i][:, t * 128:(t + 1) * 128],
                                    w2tail[i][:],
                                    start=False, stop=(i == IC - 1))
                            drain(t, nc.sync if t % 2 == 0 else nc.scalar)

            # ---- emission order: pipeline phase 1 under GEMM1 of group 0 ----
            # DMA order puts tile 0/1 activations first so the LayerNorm
            # chain (the critical path to the first matmul) starts ASAP.
            NSUB = 16   # leading GEMM1 i-chunks run as two 256-token passes
            l0 = p1_load(0, split=True)
            nc.sync.dma_start(out=bb[:, :HH], in_=bbt[:, :HH])
            nc.scalar.dma_start(out=bb[:, HH:], in_=bbt[:, HH:])
            l1 = p1_load(1, split=True)
            nc.sync.dma_start(out=ob[:], in_=obt[:])
            nc.sync.dma_start(out=b1[:], in_=b1t[:])
            make_identity(nc, ident[:])
            nc.vector.memset(eps[:], LN_EPS)
            with tc.high_priority():
                p1_compute(0, l0, split=True)
                p1_compute(1, l1)
                p1_transpose(0)
                p1_transpose(1)
            # first NSUB chunks: tokens 0-255 only (needs just tiles 0-1), so
            # the PE starts as soon as the first two LayerNorm tiles are done
            w1_first = [w1_load(i) for i in range(min(3, NSUB))]
            p1_compute(2)
            p1_compute(3)
            for i in range(0, NSUB):
                g1_chunk(0, i, w1t=w1_first[i] if i < len(w1_first) else None,
                         sub=0)
            # prefetch the re-loads for the second 256-token pass
            w1_sub1 = [w1_load(i) for i in range(2)]
            p1_transpose(2)
            p1_transpose(3)
            p1_compute(4)
            # second half of the leading chunks (tokens 256-511)
            for i in range(0, NSUB):
                g1_chunk(0, i, w1t=w1_sub1[i] if i < len(w1_sub1) else None,
                         sub=1)
            p1_compute(5)
            for i in range(NSUB, 16):
                g1_chunk(0, i)
            p1_transpose(4)
            p1_compute(6)
            for i in range(16, 24):
                g1_chunk(0, i)
            p1_transpose(5)
            p1_compute(7)
            for i in range(24, 32):
                g1_chunk(0, i)
            p1_transpose(6)
            for i in range(32, 40):
                g1_chunk(0, i)
            p1_transpose(7)
            for i in range(40, IC - 8):
                g1_chunk(0, i)
            # prefetch the first W2 column chunks (ACT queue) so GEMM2 starts
            # seamlessly after GEMM1's last chunk
            w2_first = [w2_load(0, i) for i in range(4)]
            for i in range(IC - 8, IC):
                g1_chunk(0, i)

            g2_group(0, preloaded=w2_first)
            for i in range(IC - 8):
                g1_chunk(1, i)
            w2_g1 = [w2_load(0, i) for i in range(4)]
            for i in range(IC - 8, IC):
                g1_chunk(1, i)
            g2_group(1, preloaded=w2_g1)

    nc.compile()
    return nc


def _get_program():
    if "nc" not in _CACHE:
        _CACHE["nc"] = _build_program()
    return _CACHE["nc"]


def kernel(input, residual, residual_norm, bias, gamma, beta,
           inter_w, inter_b, output_w, output_b):
    nc = _get_program()

    input = np.asarray(input, dtype=np.float32)
    residual = np.asarray(residual, dtype=np.float32)
    bias = np.asarray(bias, dtype=np.float32)
    gamma = np.asarray(gamma, dtype=np.float32)
    beta = np.asarray(beta, dtype=np.float32)
    inter_w = np.asarray(inter_w, dtype=np.float32)
    inter_b = np.asarray(inter_b, dtype=np.float32)
    output_w = np.asarray(output_w, dtype=np.float32)
    output_b = np.asarray(output_b, dtype=np.float32)

    xin = np.ascontiguousarray(input.reshape(NTOK, H).astype(NP_BF16))
    xres = np.ascontiguousarray(residual.reshape(NTOK, H).astype(NP_BF16))
    # fold LayerNorm's gamma/beta into W1/b1:
    #   ln @ W1 + b1 == z @ (gamma[:,None]*W1) + (beta @ W1 + b1)
    w1f = gamma[:, None].astype(np.float32) * inter_w
    b1f = beta.astype(np.float32) @ inter_w + inter_b
    # w1p[i, p, c*128+f] = w1f[c*128+p, i*128+f]
    w1p = np.ascontiguousarray(
        w1f.reshape(HC, 128, IC, 128).transpose(2, 1, 0, 3)
        .reshape(IC, 128, H).astype(NP_BF16))
    w2p = np.ascontiguousarray(output_w.astype(NP_BF16))
    bbt = np.ascontiguousarray(
        np.broadcast_to(bias.astype(NP_BF16), (128, H)))
    obt = np.ascontiguousarray(
        np.broadcast_to(output_b.astype(NP_BF16), (128, H)))
    b1t = np.ascontiguousarray(b1f.reshape(IC, 128).T)

    in_maps = []
    for c in range(N_CORES):
        in_maps.append({
            "xin": np.ascontiguousarray(xin[c * TLOC:(c + 1) * TLOC]),
            "xres": np.ascontiguousarray(xres[c * TLOC:(c + 1) * TLOC]),
            "w1p": w1p,
            "w2p": w2p,
            "bbt": bbt,
            "obt": obt,
            "b1t": b1t,
        })

    from concourse.bass_utils import run_bass_kernel_spmd
    res = run_bass_kernel_spmd(nc, in_maps, list(range(N_CORES)))
    out = np.concatenate([res.results[c]["out"] for c in range(N_CORES)],
                         axis=0)
    return out.reshape(B, S, H)


if __name__ == "__main__":
    nc = _get_program()
    from concourse.timeline_sim import TimelineSim
    ts = TimelineSim(nc)
    total = ts.simulate()
    print(f"TimelineSim: {total:.0f} ns")
